# revision 16
# baseline (speedup 1.0000x reference)
"""2-layer GCN on 8 Trainium2 NeuronCores (Bass/Tile).

Sharding: dst-partitioned graph parallelism. Core c owns nodes
[c*12500, (c+1)*12500) and all edges into them (~400k/core). Tiny weights
replicated. Per-layer fp16 node-feature tables are AllGathered (packed,
410KB/core) then respaced on-device into 256B-stride rows for the gather.

Math: out[d] = dinv[d]*(sum_{e->d} t[src_e] + t[d]) + b with t = h*dinv
(symmetric GCN norm factorizes; self-loop added in the node-major
epilogue for layer 1 and via B-init transposes for layer 2).

Edge machinery per core/layer:
 - padded table in DRAM: row l//4 is 256B; node l's 16 fp16 features at
   byte offset (l%4)*32. Gather: raw InstDMAGatherAnt, elem 16 fp16
   (32B), int16 row ids, round-robin on 4 SWDGE queues.
 - tiles of 128 edges target one dst window (W=43 nodes); windows are
   grouped G=24 per PSUM supergroup pacc[16, 24, 64-pitch]. Within a
   supergroup all 4 src-residue batches accumulate, then ONE ACT copy
   (layer1) / DVE add (layer2) flushes to the SBUF accumulator.
 - one-hot S^T[e-tile, dloc] built on DVE in transposed layout
   [P, W, ntiles] (broadcast on middle dim) for the 2x 16-bit fast path;
   matmul rhs reads strided column slices.
Node id l is partition-major: l = p*100 + j; host permutes x columns,
edge endpoints and output rows accordingly.
"""
import numpy as np

import concourse.bacc as bacc
import concourse.bass as bass
import concourse.mybir as mybir
import concourse.tile as tile
from concourse.bass_utils import run_bass_kernel_spmd
from concourse.masks import make_identity

P = 128
N = 100000
F_IN = 128
HID = 16
OUT = 12
C = 8
SL = 12500                 # real nodes per core
NJ = 100                   # dense tiles (p-major: l = p*NJ + j)
S = P * NJ                 # 12800 padded slice
W = 43                     # dst window width (tight pack vs 128-ceil)
NW = (S + W - 1) // W      # 298 (last window width 29)
WLAST = S - (NW - 1) * W   # 29
G = 16                     # windows per PSUM supergroup
NPG = (NW + G - 1) // G    # 13 (last pg has 10 windows)
NRES = 4                   # table packing: 4 nodes / 256B row
ROWC = 128                 # fp16 elems per padded table row (64 used)
PACKC = NRES * HID         # 64 packed fp16 elems per row
ROWS_L = S // NRES         # 3200
ROWS_G = ROWS_L * C        # 25600  (int16-safe)
DT = mybir.dt


def raw_gather(nc, out_ap, in_ap, idxs_ap, num_idxs, queue_num):
    gp = nc.gpsimd
    _in_ap = gp.lower_ap_dma(in_ap, for_custom_bir_dma=True)
    _idxs_ap = gp.lower_ap(idxs_ap)
    _out_ap = gp.lower_ap(out_ap)
    return gp.add_instruction(mybir.InstDMAGatherAnt(
        name=nc.get_next_instruction_name(),
        ins=[*_in_ap, _idxs_ap, gp.lower_val_access(gp.to_reg(num_idxs))],
        outs=[_out_ap],
        transpose=False,
        num_idxs=num_idxs,
        elem_size=HID,
        stride_bytes_256=1,
        gen_mode=0,
        single_packet=False,
        queue_num=queue_num,
    ))


def pg_structure(ntile_rw):
    """Global tile order: (pg, r, w, k). Returns per-pg info."""
    pgs = []
    t0 = 0
    for pg in range(NPG):
        w0, w1 = pg * G, min((pg + 1) * G, NW)
        rinfo = []
        for r in range(NRES):
            cnt = int(sum(ntile_rw[r][w] for w in range(w0, w1)))
            rinfo.append((t0, cnt))
            t0 += cnt
        pgs.append((w0, w1, rinfo))
    return pgs, t0


def build_program(ntile_rw, n_slots):
    pgs, nt = pg_structure(ntile_rw)
    MAXT = max(cnt for (_, _, ri) in pgs for (_, cnt) in ri)
    MAXPG = max(sum(c for (_, c) in ri) for (_, _, ri) in pgs)

    nc = bacc.Bacc("TRN2", target_bir_lowering=False, debug=False,
                   num_devices=C, num_swdge_queues=4)

    xT = nc.dram_tensor("xT", [F_IN, S], DT.float16, kind="ExternalInput")
    w1 = nc.dram_tensor("w1", [F_IN, HID], DT.float16, kind="ExternalInput")
    b1 = nc.dram_tensor("b1", [1, HID], DT.float32, kind="ExternalInput")
    w2 = nc.dram_tensor("w2", [HID, OUT], DT.float32, kind="ExternalInput")
    b2 = nc.dram_tensor("b2", [1, OUT], DT.float32, kind="ExternalInput")
    deg_in = nc.dram_tensor("deg", [P, NJ], DT.float32, kind="ExternalInput")
    idx_in = nc.dram_tensor("idx", [P, n_slots // 16], DT.int16, kind="ExternalInput")
    dstw_in = nc.dram_tensor("dstw", [P, nt], DT.float16, kind="ExternalInput")
    out_t = nc.dram_tensor("out", [P, NJ * OUT], DT.float32, kind="ExternalOutput")

    with tile.TileContext(nc) as tc:
        with tc.tile_pool(name="con", bufs=1) as con, \
             tc.tile_pool(name="dram", bufs=1, space="DRAM") as dpool, \
             tc.tile_pool(name="sb", bufs=3) as sb, \
             tc.tile_pool(name="gat", bufs=9) as gat, \
             tc.tile_pool(name="ohp", bufs=9) as ohp, \
             tc.tile_pool(name="ps", bufs=2, space="PSUM") as ps, \
             tc.tile_pool(name="pst", bufs=2, space="PSUM") as pst:

            iota_i = con.tile([P, W], DT.int32)
            nc.gpsimd.iota(iota_i[:], pattern=[[1, W]], base=0, channel_multiplier=0)
            iota_f = con.tile([P, W], DT.float16)
            nc.vector.tensor_copy(out=iota_f[:], in_=iota_i[:])
            iota_rep = con.tile([P, W, MAXT], DT.float16)
            nc.vector.tensor_copy(
                out=iota_rep[:],
                in_=iota_f[:][:, :, None].broadcast_to([P, W, MAXT]))
            ident32 = con.tile([HID, HID], DT.float32)
            make_identity(nc, ident32[:])
            ident_p = con.tile([P, P], DT.float32)
            make_identity(nc, ident_p[:])

            w1_sb = con.tile([F_IN, HID], DT.float16)
            nc.sync.dma_start(out=w1_sb[:], in_=w1[:])
            w2_sb = con.tile([HID, OUT], DT.float32)
            nc.sync.dma_start(out=w2_sb[:], in_=w2[:])
            b1row = con.tile([1, HID], DT.float32)
            nc.sync.dma_start(out=b1row[:], in_=b1[:])
            b1b = con.tile([P, HID], DT.float32)
            nc.gpsimd.partition_broadcast(b1b[:], b1row[:], channels=P)
            b2row = con.tile([1, OUT], DT.float32)
            nc.sync.dma_start(out=b2row[:], in_=b2[:])
            b2b = con.tile([P, OUT], DT.float32)
            nc.gpsimd.partition_broadcast(b2b[:], b2row[:], channels=P)

            deg = con.tile([P, NJ], DT.float32)
            nc.sync.dma_start(out=deg[:], in_=deg_in[:])
            dinv_r = con.tile([P, NJ], DT.float32)
            nc.vector.reciprocal(out=dinv_r[:], in_=deg[:])
            dinv = con.tile([P, NJ], DT.float32)
            nc.scalar.activation(out=dinv[:], in_=dinv_r[:],
                                 func=mybir.ActivationFunctionType.Sqrt)
            A = con.tile([HID, S], DT.float32)
            B = con.tile([HID, S], DT.float32)

            dstw_sb = con.tile([P, nt], DT.float16)
            nc.sync.dma_start(out=dstw_sb[:], in_=dstw_in[:])

            # packed AG buffers + padded gather tables
            t1_loc = dpool.tile([ROWS_L, PACKC], DT.float16)
            t1_gp = dpool.tile([ROWS_G, PACKC], DT.float16, addr_space="Shared")
            t1_pad = dpool.tile([ROWS_G, ROWC], DT.float16)
            t2_loc = dpool.tile([ROWS_L, PACKC], DT.float16)
            t2_gp = dpool.tile([ROWS_G, PACKC], DT.float16, addr_space="Shared")
            t2_pad = dpool.tile([ROWS_G, ROWC], DT.float16)

            h1s = con.tile([P, NJ, HID], DT.float16)
            u_st = con.tile([P, NJ, HID], DT.float32)
            u2 = con.tile([P, NJ, HID], DT.float32)
            u_nm = con.tile([P, NJ, HID], DT.float16)

            _scope = [None, None]

            def mark(name):
                if _scope[0] is not None:
                    nc.leave_named_scope(_scope[0], _scope[1], False)
                    _scope[0] = None
                if name is not None:
                    sid, _ = nc.enter_named_scope(name, False)
                    _scope[0], _scope[1] = name, sid

            def respace(dst_pad, src_packed):
                half = ROWS_G // 2
                for k in range(2):
                    nc.sync.dma_start(
                        out=dst_pad[k * half:(k + 1) * half, :PACKC],
                        in_=src_packed[k * half:(k + 1) * half, :])

            # ---- dense: h1s[p, j, :] = (x @ W1) * dinv, x cols (j, p) ----
            mark("dense1")
            for j0 in range(0, NJ, 4):
                xt = sb.tile([F_IN, 4 * P], DT.float16, tag="xt")
                nc.sync.dma_start(out=xt[:], in_=xT[:, j0 * P:(j0 + 4) * P])
                pd4 = pst.tile([P, 4, HID], DT.float32, space="PSUM", tag="pp")
                for jj in range(4):
                    nc.tensor.matmul(out=pd4[:, jj, :],
                                     lhsT=xt[:, jj * P:(jj + 1) * P],
                                     rhs=w1_sb[:], start=True, stop=True)
                nc.vector.tensor_tensor(
                    out=h1s[:, j0:j0 + 4, :], in0=pd4[:],
                    in1=dinv[:, j0:j0 + 4][:, :, None].broadcast_to([P, 4, HID]),
                    op=mybir.AluOpType.mult)

            # table1 = h1s image (row l//4 = p*25 + j//4), packed -> AG -> pad
            mark("ag1")
            nc.sync.dma_start(
                out=t1_loc[:].rearrange("(p j4) c -> p j4 c", p=P),
                in_=h1s[:].rearrange("p (j4 jr) f -> p j4 (jr f)", jr=NRES))
            nc.gpsimd.collective_compute(
                "AllGather", mybir.AluOpType.bypass,
                replica_groups=[list(range(C))],
                ins=[t1_loc[:]], outs=[t1_gp[:]])
            respace(t1_pad, t1_gp)

            def edge_phase(tpad, acc, idxpool, init_copy):
                for pg, (w0, w1_, rinfo) in enumerate(pgs):
                    gw = w1_ - w0
                    pg_t0 = rinfo[0][0]
                    pg_nt = sum(c for (_, c) in rinfo)
                    idx_ch = idxpool.tile([P, MAXPG * 8], DT.int16, tag="idxch")
                    nc.sync.dma_start(
                        out=idx_ch[:, :pg_nt * 8],
                        in_=idx_in[:, pg_t0 * 8:(pg_t0 + pg_nt) * 8])
                    dests, ohs = [], []
                    for r in range(NRES):
                        t0, cnt = rinfo[r]
                        dest = gat.tile([P, MAXT, HID], DT.float16, tag="gd")
                        off = (t0 - pg_t0) * 8
                        raw_gather(nc, dest[:, :cnt, :],
                                   tpad[:, r * HID:(r + 1) * HID],
                                   idx_ch[:, off:off + cnt * 8],
                                   cnt * P, queue_num=(pg * NRES + r) % 4)
                        oh = ohp.tile([P, W, MAXT], DT.float16, tag="oh")
                        nc.vector.tensor_tensor(
                            out=oh[:, :, :cnt],
                            in0=dstw_sb[:, t0:t0 + cnt][:, None, :]
                                .broadcast_to([P, W, cnt]),
                            in1=iota_rep[:, :, :cnt],
                            op=mybir.AluOpType.is_equal)
                        dests.append(dest)
                        ohs.append(oh)
                    pacc = ps.tile([HID, G, 64], DT.float32, space="PSUM",
                                   tag="pacc")
                    # w-major: each window's accumulation group stays
                    # consecutive on the PE (interleaved groups corrupt
                    # PSUM: only the last group per bank survives)
                    woffs = [0] * NRES
                    for wi, w in enumerate(range(w0, w1_)):
                        for r in range(NRES):
                            T = int(ntile_rw[r][w])
                            for k in range(T):
                                c = woffs[r] + k
                                nc.tensor.matmul(
                                    out=pacc[:, wi, :W],
                                    lhsT=dests[r][:, c, :],
                                    rhs=ohs[r][:, :, c],
                                    start=(r == 0 and k == 0),
                                    stop=(r == NRES - 1 and k == T - 1),
                                    skip_group_check=True)
                            woffs[r] += T
                    # flush supergroup
                    a0 = w0 * W
                    full = gw if w1_ < NW else gw - 1
                    accv = acc[:, a0:a0 + full * W].rearrange(
                        "f (g w) -> f g w", w=W)
                    if init_copy:
                        if full:
                            nc.scalar.activation(
                                out=accv, in_=pacc[:, :full, :W],
                                func=mybir.ActivationFunctionType.Copy)
                        if w1_ == NW:
                            nc.scalar.activation(
                                out=acc[:, a0 + full * W:S],
                                in_=pacc[:, full, :WLAST],
                                func=mybir.ActivationFunctionType.Copy)
                    else:
                        if full:
                            nc.vector.tensor_tensor(
                                out=accv, in0=accv,
                                in1=pacc[:, :full, :W],
                                op=mybir.AluOpType.add)
                        if w1_ == NW:
                            nc.vector.tensor_tensor(
                                out=acc[:, a0 + full * W:S],
                                in0=acc[:, a0 + full * W:S],
                                in1=pacc[:, full, :WLAST],
                                op=mybir.AluOpType.add)

            with tc.tile_pool(name="idxp", bufs=2) as idxpool:
                mark("edge1")
                edge_phase(t1_pad, A[:], idxpool, init_copy=True)
                mark("epi1")

                # L1 epilogue node-major: u2 = relu((A^T + h1s)*dinv + b1)*dinv
                for j0 in range(0, NJ, 4):
                    pu4 = pst.tile([P, 4, HID], DT.float32, space="PSUM",
                                   tag="pp")
                    for jj in range(4):
                        nc.tensor.transpose(
                            out=pu4[:, jj, :],
                            in_=A[:].rearrange("f (p jj) -> f p jj",
                                               jj=NJ)[:, :, j0 + jj],
                            identity=ident32[:])
                    nc.scalar.activation(out=u_st[:, j0:j0 + 4, :],
                                         in_=pu4[:],
                                         func=mybir.ActivationFunctionType.Copy)
                nc.vector.tensor_tensor(
                    out=u_st[:], in0=u_st[:], in1=h1s[:],
                    op=mybir.AluOpType.add)
                nc.vector.tensor_tensor(
                    out=u_st[:], in0=u_st[:],
                    in1=dinv[:][:, :, None].broadcast_to([P, NJ, HID]),
                    op=mybir.AluOpType.mult)
                nc.vector.tensor_tensor(
                    out=u_st[:], in0=u_st[:],
                    in1=b1b[:][:, None, :].broadcast_to([P, NJ, HID]),
                    op=mybir.AluOpType.add)
                nc.scalar.activation(out=u_st[:], in_=u_st[:],
                                     func=mybir.ActivationFunctionType.Relu)
                nc.vector.tensor_tensor(
                    out=u2[:], in0=u_st[:],
                    in1=dinv[:][:, :, None].broadcast_to([P, NJ, HID]),
                    op=mybir.AluOpType.mult)
                nc.vector.tensor_copy(out=u_nm[:], in_=u2[:])
                mark("ag2")
                nc.sync.dma_start(
                    out=t2_loc[:].rearrange("(p j4) c -> p j4 c", p=P),
                    in_=u_nm[:].rearrange("p (j4 jr) f -> p j4 (jr f)",
                                          jr=NRES))
                nc.gpsimd.collective_compute(
                    "AllGather", mybir.AluOpType.bypass,
                    replica_groups=[list(range(C))],
                    ins=[t2_loc[:]], outs=[t2_gp[:]])
                respace(t2_pad, t2_gp)
                # B init = u2^T (self-loop term for layer 2)
                for j0 in range(0, NJ, 4):
                    pb4 = pst.tile([HID, 4, P], DT.float32, space="PSUM",
                                   tag="pp2")
                    for jj in range(4):
                        nc.tensor.transpose(out=pb4[:, jj, :],
                                            in_=u2[:, j0 + jj, :],
                                            identity=ident_p[:])
                    nc.scalar.activation(
                        out=B[:].rearrange("f (p jj) -> f jj p", jj=NJ)[
                            :, j0:j0 + 4, :],
                        in_=pb4[:], func=mybir.ActivationFunctionType.Copy)
                mark("edge2")
                edge_phase(t2_pad, B[:], idxpool, init_copy=False)
                mark("final")

            # L2 final: y = (B^T @ W2)*dinv + b2 ; log_softmax
            y = con.tile([P, NJ, OUT], DT.float32)
            for j0 in range(0, NJ, 4):
                py4 = pst.tile([P, 4, OUT], DT.float32, space="PSUM", tag="pp")
                for jj in range(4):
                    nc.tensor.matmul(
                        out=py4[:, jj, :],
                        lhsT=B[:].rearrange("f (p jj) -> f p jj",
                                            jj=NJ)[:, :, j0 + jj],
                        rhs=w2_sb[:], start=True, stop=True)
                nc.scalar.activation(out=y[:, j0:j0 + 4, :], in_=py4[:],
                                     func=mybir.ActivationFunctionType.Copy)
            nc.vector.tensor_tensor(
                out=y[:], in0=y[:],
                in1=dinv[:][:, :, None].broadcast_to([P, NJ, OUT]),
                op=mybir.AluOpType.mult)
            nc.vector.tensor_tensor(
                out=y[:], in0=y[:],
                in1=b2b[:][:, None, :].broadcast_to([P, NJ, OUT]),
                op=mybir.AluOpType.add)
            mx = con.tile([P, NJ], DT.float32)
            nc.vector.tensor_reduce(out=mx[:], in_=y[:], op=mybir.AluOpType.max,
                                    axis=mybir.AxisListType.X)
            ys = con.tile([P, NJ, OUT], DT.float32)
            nc.vector.tensor_tensor(
                out=ys[:], in0=y[:],
                in1=mx[:][:, :, None].broadcast_to([P, NJ, OUT]),
                op=mybir.AluOpType.subtract)
            ex = u_st[:, :, :OUT]  # reuse
            nc.scalar.activation(out=ex,
                                 in_=ys[:], func=mybir.ActivationFunctionType.Exp)
            sm = con.tile([P, NJ], DT.float32)
            nc.vector.tensor_reduce(out=sm[:], in_=ex, op=mybir.AluOpType.add,
                                    axis=mybir.AxisListType.X)
            lg = con.tile([P, NJ], DT.float32)
            nc.scalar.activation(out=lg[:], in_=sm[:],
                                 func=mybir.ActivationFunctionType.Ln)
            res = y  # reuse
            nc.vector.tensor_tensor(
                out=res[:], in0=ys[:],
                in1=lg[:][:, :, None].broadcast_to([P, NJ, OUT]),
                op=mybir.AluOpType.subtract)
            nc.sync.dma_start(out=out_t[:], in_=res[:].rearrange("p j o -> p (j o)"))
            mark(None)

    nc.compile()
    return nc


def prepare(x, edge_index, W1, b1v, W2, b2v):
    x = np.asarray(x, np.float32)
    ei = np.asarray(edge_index)
    src = ei[0].astype(np.int64)
    dst = ei[1].astype(np.int64)

    core_data = []
    for c in range(C):
        m = (dst >= c * SL) & (dst < (c + 1) * SL)
        s_c = src[m]
        d_c = dst[m] - c * SL          # local dst id l in [0, 12500)
        res = s_c % NRES               # src%4 (12500%4==0 so local==global)
        win = d_c // W
        pg = win // G
        order = np.lexsort((d_c, win, res, pg))
        core_data.append((s_c[order], d_c[order],
                          res[order], win[order]))

    ntile_rw = np.zeros((NRES, NW), np.int64)
    for c in range(C):
        _, d_c, r_c, w_c = core_data[c]
        for r in range(NRES):
            wcounts = np.bincount(w_c[r_c == r], minlength=NW)
            ntile_rw[r] = np.maximum(ntile_rw[r], (wcounts + P - 1) // P)

    pgs, nt = pg_structure(ntile_rw)
    n_slots = nt * P

    in_maps = []
    for c in range(C):
        s_c, d_c, r_c, w_c = core_data[c]
        idx_flat = np.zeros(n_slots, np.int16)
        dstw = np.full((P, nt), 512.0, np.float32)
        # edges sorted by (pg, r, w, d); groups keyed (pg, r, w)
        pg_c = w_c // G
        grp_id = (pg_c * NRES + r_c) * NW + w_c
        bounds = np.searchsorted(grp_id, np.arange(NPG * NRES * NW + 1))
        t0 = 0
        for pg in range(NPG):
            w0, w1_ = pg * G, min((pg + 1) * G, NW)
            for r in range(NRES):
                for w in range(w0, w1_):
                    g = (pg * NRES + r) * NW + w
                    lo, hi = bounds[g], bounds[g + 1]
                    cnt = hi - lo
                    T = int(ntile_rw[r, w])
                    rows = ((s_c[lo:hi] // SL) * ROWS_L
                            + (s_c[lo:hi] % SL) // NRES)
                    dloc = d_c[lo:hi] - w * W
                    sl = np.zeros(T * P, np.int64)
                    dw = np.full(T * P, 512.0, np.float32)
                    sl[:cnt] = rows
                    dw[:cnt] = dloc
                    idx_flat[t0 * P:(t0 + T) * P] = sl.astype(np.int16)
                    dstw[:, t0:t0 + T] = dw.reshape(T, P).T
                    t0 += T
        assert t0 == nt
        idx_wrapped = np.tile(idx_flat.reshape(n_slots // 16, 16).T, (8, 1)).copy()

        xs = np.zeros((S, F_IN), np.float32)
        xs[:SL] = x[c * SL:(c + 1) * SL]
        # xT columns ordered (j, p) with node l = p*NJ + j
        xT = np.ascontiguousarray(
            xs.reshape(P, NJ, F_IN).transpose(2, 1, 0).reshape(F_IN, S)
        ).astype(np.float16)

        degc = np.bincount(d_c, minlength=S).astype(np.float32) + 1.0
        deg_pj = degc.reshape(P, NJ).copy()            # [p, j] = deg[p*NJ+j]

        in_maps.append({
            "xT": xT,
            "w1": np.asarray(W1, np.float16),
            "b1": np.asarray(b1v, np.float32).reshape(1, HID),
            "w2": np.asarray(W2, np.float32),
            "b2": np.asarray(b2v, np.float32).reshape(1, OUT),
            "deg": deg_pj,
            "idx": idx_wrapped,
            "dstw": dstw.astype(np.float16),
        })
    return ntile_rw, n_slots, in_maps


def kernel(x, edge_index, W1, b1, W2, b2):
    ntile_rw, n_slots, in_maps = prepare(x, edge_index, W1, b1, W2, b2)
    nc = build_program(ntile_rw, n_slots)
    res = run_bass_kernel_spmd(nc, in_maps, core_ids=list(range(C)))
    outs = []
    for c in range(C):
        o = res.results[c]["out"].reshape(P, NJ, OUT)   # [p, j, o], l = p*NJ+j
        o = o.reshape(S, OUT)[:SL]
        outs.append(o)
    return np.concatenate(outs, 0).astype(np.float32)


# revision 17
# speedup vs baseline: 1.0001x; 1.0001x over previous
"""2-layer GCN on 8 Trainium2 NeuronCores (Bass/Tile).

Sharding: dst-partitioned graph parallelism. Core c owns nodes
[c*12500, (c+1)*12500) and all edges into them (~400k/core). Tiny weights
replicated. Per-layer fp16 node-feature tables are AllGathered (packed,
410KB/core) then respaced on-device into 256B-stride rows for the gather.

Math: out[d] = dinv[d]*(sum_{e->d} t[src_e] + t[d]) + b with t = h*dinv
(symmetric GCN norm factorizes; self-loop added in the node-major
epilogue for layer 1 and via B-init transposes for layer 2).

Edge machinery per core/layer:
 - padded table in DRAM: row l//4 is 256B; node l's 16 fp16 features at
   byte offset (l%4)*32. Gather: raw InstDMAGatherAnt, elem 16 fp16
   (32B), int16 row ids, round-robin on 4 SWDGE queues.
 - tiles of 128 edges target one dst window (W=43 nodes); windows are
   grouped G=24 per PSUM supergroup pacc[16, 24, 64-pitch]. Within a
   supergroup all 4 src-residue batches accumulate, then ONE ACT copy
   (layer1) / DVE add (layer2) flushes to the SBUF accumulator.
 - one-hot S^T[e-tile, dloc] built on DVE in transposed layout
   [P, W, ntiles] (broadcast on middle dim) for the 2x 16-bit fast path;
   matmul rhs reads strided column slices.
Node id l is partition-major: l = p*100 + j; host permutes x columns,
edge endpoints and output rows accordingly.
"""
import numpy as np

import concourse.bacc as bacc
import concourse.bass as bass
import concourse.mybir as mybir
import concourse.tile as tile
from concourse.bass_utils import run_bass_kernel_spmd
from concourse.masks import make_identity

P = 128
N = 100000
F_IN = 128
HID = 16
OUT = 12
C = 8
SL = 12500                 # real nodes per core
NJ = 100                   # dense tiles (p-major: l = p*NJ + j)
S = P * NJ                 # 12800 padded slice
W = 43                     # dst window width (tight pack vs 128-ceil)
NW = (S + W - 1) // W      # 298 (last window width 29)
WLAST = S - (NW - 1) * W   # 29
G = 16                     # windows per PSUM supergroup
NPG = (NW + G - 1) // G    # 13 (last pg has 10 windows)
NRES = 4                   # table packing: 4 nodes / 256B row
ROWC = 256                 # fp8 elems per padded 256B table row (64 used)
PACKC = NRES * HID         # 64 packed fp8 elems (bytes) per row
ROWS_L = S // NRES         # 3200
ROWS_G = ROWS_L * C        # 25600  (int16-safe)
DT = mybir.dt


def raw_gather(nc, out_ap, in_ap, idxs_ap, num_idxs, queue_num):
    gp = nc.gpsimd
    _in_ap = gp.lower_ap_dma(in_ap, for_custom_bir_dma=True)
    _idxs_ap = gp.lower_ap(idxs_ap)
    _out_ap = gp.lower_ap(out_ap)
    return gp.add_instruction(mybir.InstDMAGatherAnt(
        name=nc.get_next_instruction_name(),
        ins=[*_in_ap, _idxs_ap, gp.lower_val_access(gp.to_reg(num_idxs))],
        outs=[_out_ap],
        transpose=False,
        num_idxs=num_idxs,
        elem_size=HID,
        stride_bytes_256=1,
        gen_mode=0,
        single_packet=False,
        queue_num=queue_num,
    ))


def pg_structure(ntile_rw):
    """Global tile order: (pg, r, w, k). Returns per-pg info."""
    pgs = []
    t0 = 0
    for pg in range(NPG):
        w0, w1 = pg * G, min((pg + 1) * G, NW)
        rinfo = []
        for r in range(NRES):
            cnt = int(sum(ntile_rw[r][w] for w in range(w0, w1)))
            rinfo.append((t0, cnt))
            t0 += cnt
        pgs.append((w0, w1, rinfo))
    return pgs, t0


def build_program(ntile_rw, n_slots):
    pgs, nt = pg_structure(ntile_rw)
    MAXT = max(cnt for (_, _, ri) in pgs for (_, cnt) in ri)
    MAXPG = max(sum(c for (_, c) in ri) for (_, _, ri) in pgs)

    nc = bacc.Bacc("TRN2", target_bir_lowering=False, debug=False,
                   num_devices=C, num_swdge_queues=4)

    xT = nc.dram_tensor("xT", [F_IN, S], DT.float16, kind="ExternalInput")
    w1 = nc.dram_tensor("w1", [F_IN, HID], DT.float16, kind="ExternalInput")
    b1 = nc.dram_tensor("b1", [1, HID], DT.float32, kind="ExternalInput")
    w2 = nc.dram_tensor("w2", [HID, OUT], DT.float32, kind="ExternalInput")
    b2 = nc.dram_tensor("b2", [1, OUT], DT.float32, kind="ExternalInput")
    deg_in = nc.dram_tensor("deg", [P, NJ], DT.float32, kind="ExternalInput")
    idx_in = nc.dram_tensor("idx", [P, n_slots // 16], DT.int16, kind="ExternalInput")
    dstw_in = nc.dram_tensor("dstw", [P, nt], DT.float16, kind="ExternalInput")
    out_t = nc.dram_tensor("out", [P, NJ * OUT], DT.float32, kind="ExternalOutput")

    with tile.TileContext(nc) as tc:
        with tc.tile_pool(name="con", bufs=1) as con, \
             tc.tile_pool(name="dram", bufs=1, space="DRAM") as dpool, \
             tc.tile_pool(name="sb", bufs=3) as sb, \
             tc.tile_pool(name="gat", bufs=9) as gat, \
             tc.tile_pool(name="ohp", bufs=9) as ohp, \
             tc.tile_pool(name="ps", bufs=2, space="PSUM") as ps, \
             tc.tile_pool(name="pst", bufs=2, space="PSUM") as pst:

            iota_i = con.tile([P, W], DT.int32)
            nc.gpsimd.iota(iota_i[:], pattern=[[1, W]], base=0, channel_multiplier=0)
            iota_f = con.tile([P, W], DT.float16)
            nc.vector.tensor_copy(out=iota_f[:], in_=iota_i[:])
            iota_rep = con.tile([P, W, MAXT], DT.float16)
            nc.vector.tensor_copy(
                out=iota_rep[:],
                in_=iota_f[:][:, :, None].broadcast_to([P, W, MAXT]))
            ident32 = con.tile([HID, HID], DT.float32)
            make_identity(nc, ident32[:])
            ident_p = con.tile([P, P], DT.float32)
            make_identity(nc, ident_p[:])

            w1_sb = con.tile([F_IN, HID], DT.float16)
            nc.sync.dma_start(out=w1_sb[:], in_=w1[:])
            w2_sb = con.tile([HID, OUT], DT.float32)
            nc.sync.dma_start(out=w2_sb[:], in_=w2[:])
            b1row = con.tile([1, HID], DT.float32)
            nc.sync.dma_start(out=b1row[:], in_=b1[:])
            b1b = con.tile([P, HID], DT.float32)
            nc.gpsimd.partition_broadcast(b1b[:], b1row[:], channels=P)
            b2row = con.tile([1, OUT], DT.float32)
            nc.sync.dma_start(out=b2row[:], in_=b2[:])
            b2b = con.tile([P, OUT], DT.float32)
            nc.gpsimd.partition_broadcast(b2b[:], b2row[:], channels=P)

            deg = con.tile([P, NJ], DT.float32)
            nc.sync.dma_start(out=deg[:], in_=deg_in[:])
            dinv_r = con.tile([P, NJ], DT.float32)
            nc.vector.reciprocal(out=dinv_r[:], in_=deg[:])
            dinv = con.tile([P, NJ], DT.float32)
            nc.scalar.activation(out=dinv[:], in_=dinv_r[:],
                                 func=mybir.ActivationFunctionType.Sqrt)
            A = con.tile([HID, S], DT.float32)
            B = con.tile([HID, S], DT.float32)

            dstw_sb = con.tile([P, nt], DT.float16)
            nc.sync.dma_start(out=dstw_sb[:], in_=dstw_in[:])

            # packed AG buffers + padded gather tables
            t1_loc = dpool.tile([ROWS_L, PACKC], DT.float8e4)
            t1_gp = dpool.tile([ROWS_G, PACKC], DT.float8e4, addr_space="Shared")
            t1_pad = dpool.tile([ROWS_G, ROWC], DT.float8e4)
            t2_loc = dpool.tile([ROWS_L, PACKC], DT.float8e4)
            t2_gp = dpool.tile([ROWS_G, PACKC], DT.float8e4, addr_space="Shared")
            t2_pad = dpool.tile([ROWS_G, ROWC], DT.float8e4)

            h1s = con.tile([P, NJ, HID], DT.float16)
            h1s_f8 = con.tile([P, NJ, HID], DT.float8e4)
            u_f8 = con.tile([P, NJ, HID], DT.float8e4)
            u_st = con.tile([P, NJ, HID], DT.float32)
            u2 = con.tile([P, NJ, HID], DT.float32)

            _scope = [None, None]

            def mark(name):
                if _scope[0] is not None:
                    nc.leave_named_scope(_scope[0], _scope[1], False)
                    _scope[0] = None
                if name is not None:
                    sid, _ = nc.enter_named_scope(name, False)
                    _scope[0], _scope[1] = name, sid

            def respace(dst_pad, src_packed):
                half = ROWS_G // 2
                for k in range(2):
                    nc.sync.dma_start(
                        out=dst_pad[k * half:(k + 1) * half, :PACKC],
                        in_=src_packed[k * half:(k + 1) * half, :])

            # ---- dense: h1s[p, j, :] = (x @ W1) * dinv, x cols (j, p) ----
            mark("dense1")
            for j0 in range(0, NJ, 4):
                xt = sb.tile([F_IN, 4 * P], DT.float16, tag="xt")
                nc.sync.dma_start(out=xt[:], in_=xT[:, j0 * P:(j0 + 4) * P])
                pd4 = pst.tile([P, 4, HID], DT.float32, space="PSUM", tag="pp")
                for jj in range(4):
                    nc.tensor.matmul(out=pd4[:, jj, :],
                                     lhsT=xt[:, jj * P:(jj + 1) * P],
                                     rhs=w1_sb[:], start=True, stop=True)
                nc.vector.tensor_tensor(
                    out=h1s[:, j0:j0 + 4, :], in0=pd4[:],
                    in1=dinv[:, j0:j0 + 4][:, :, None].broadcast_to([P, 4, HID]),
                    op=mybir.AluOpType.mult)

            # table1 = h1s image (row l//4 = p*25 + j//4), packed -> AG -> pad
            mark("ag1")
            nc.vector.tensor_copy(out=h1s_f8[:], in_=h1s[:])
            nc.sync.dma_start(
                out=t1_loc[:].rearrange("(p j4) c -> p j4 c", p=P),
                in_=h1s_f8[:].rearrange("p (j4 jr) f -> p j4 (jr f)", jr=NRES))
            nc.gpsimd.collective_compute(
                "AllGather", mybir.AluOpType.bypass,
                replica_groups=[list(range(C))],
                ins=[t1_loc[:]], outs=[t1_gp[:]])
            respace(t1_pad, t1_gp)

            def edge_phase(tpad, acc, idxpool, init_copy):
                for pg, (w0, w1_, rinfo) in enumerate(pgs):
                    gw = w1_ - w0
                    pg_t0 = rinfo[0][0]
                    pg_nt = sum(c for (_, c) in rinfo)
                    idx_ch = idxpool.tile([P, MAXPG * 8], DT.int16, tag="idxch")
                    nc.sync.dma_start(
                        out=idx_ch[:, :pg_nt * 8],
                        in_=idx_in[:, pg_t0 * 8:(pg_t0 + pg_nt) * 8])
                    dests, ohs = [], []
                    for r in range(NRES):
                        t0, cnt = rinfo[r]
                        dest = gat.tile([P, MAXT, HID], DT.float8e4, tag="gd")
                        off = (t0 - pg_t0) * 8
                        raw_gather(nc, dest[:, :cnt, :],
                                   tpad[:, r * HID:(r + 1) * HID],
                                   idx_ch[:, off:off + cnt * 8],
                                   cnt * P, queue_num=(pg * NRES + r) % 4)
                        oh = ohp.tile([P, W, MAXT], DT.float16, tag="oh")
                        nc.vector.tensor_tensor(
                            out=oh[:, :, :cnt],
                            in0=dstw_sb[:, t0:t0 + cnt][:, None, :]
                                .broadcast_to([P, W, cnt]),
                            in1=iota_rep[:, :, :cnt],
                            op=mybir.AluOpType.is_equal)
                        dests.append(dest)
                        ohs.append(oh)
                    pacc = ps.tile([HID, G, 64], DT.float32, space="PSUM",
                                   tag="pacc")
                    # w-major: each window's accumulation group stays
                    # consecutive on the PE (interleaved groups corrupt
                    # PSUM: only the last group per bank survives)
                    woffs = [0] * NRES
                    for wi, w in enumerate(range(w0, w1_)):
                        for r in range(NRES):
                            T = int(ntile_rw[r][w])
                            for k in range(T):
                                c = woffs[r] + k
                                nc.tensor.matmul(
                                    out=pacc[:, wi, :W],
                                    lhsT=dests[r][:, c, :],
                                    rhs=ohs[r][:, :, c],
                                    start=(r == 0 and k == 0),
                                    stop=(r == NRES - 1 and k == T - 1),
                                    skip_group_check=True)
                            woffs[r] += T
                    # flush supergroup
                    a0 = w0 * W
                    full = gw if w1_ < NW else gw - 1
                    accv = acc[:, a0:a0 + full * W].rearrange(
                        "f (g w) -> f g w", w=W)
                    if init_copy:
                        if full:
                            nc.scalar.activation(
                                out=accv, in_=pacc[:, :full, :W],
                                func=mybir.ActivationFunctionType.Copy)
                        if w1_ == NW:
                            nc.scalar.activation(
                                out=acc[:, a0 + full * W:S],
                                in_=pacc[:, full, :WLAST],
                                func=mybir.ActivationFunctionType.Copy)
                    else:
                        if full:
                            nc.vector.tensor_tensor(
                                out=accv, in0=accv,
                                in1=pacc[:, :full, :W],
                                op=mybir.AluOpType.add)
                        if w1_ == NW:
                            nc.vector.tensor_tensor(
                                out=acc[:, a0 + full * W:S],
                                in0=acc[:, a0 + full * W:S],
                                in1=pacc[:, full, :WLAST],
                                op=mybir.AluOpType.add)

            with tc.tile_pool(name="idxp", bufs=2) as idxpool:
                mark("edge1")
                edge_phase(t1_pad, A[:], idxpool, init_copy=True)
                mark("epi1")

                # L1 epilogue node-major: u2 = relu((A^T + h1s)*dinv + b1)*dinv
                for j0 in range(0, NJ, 4):
                    pu4 = pst.tile([P, 4, HID], DT.float32, space="PSUM",
                                   tag="pp")
                    for jj in range(4):
                        nc.tensor.transpose(
                            out=pu4[:, jj, :],
                            in_=A[:].rearrange("f (p jj) -> f p jj",
                                               jj=NJ)[:, :, j0 + jj],
                            identity=ident32[:])
                    nc.scalar.activation(out=u_st[:, j0:j0 + 4, :],
                                         in_=pu4[:],
                                         func=mybir.ActivationFunctionType.Copy)
                nc.vector.tensor_tensor(
                    out=u_st[:], in0=u_st[:], in1=h1s[:],
                    op=mybir.AluOpType.add)
                nc.vector.tensor_tensor(
                    out=u_st[:], in0=u_st[:],
                    in1=dinv[:][:, :, None].broadcast_to([P, NJ, HID]),
                    op=mybir.AluOpType.mult)
                nc.vector.tensor_tensor(
                    out=u_st[:], in0=u_st[:],
                    in1=b1b[:][:, None, :].broadcast_to([P, NJ, HID]),
                    op=mybir.AluOpType.add)
                nc.scalar.activation(out=u_st[:], in_=u_st[:],
                                     func=mybir.ActivationFunctionType.Relu)
                nc.vector.tensor_tensor(
                    out=u2[:], in0=u_st[:],
                    in1=dinv[:][:, :, None].broadcast_to([P, NJ, HID]),
                    op=mybir.AluOpType.mult)
                mark("ag2")
                nc.vector.tensor_copy(out=u_f8[:], in_=u2[:])
                nc.sync.dma_start(
                    out=t2_loc[:].rearrange("(p j4) c -> p j4 c", p=P),
                    in_=u_f8[:].rearrange("p (j4 jr) f -> p j4 (jr f)",
                                          jr=NRES))
                nc.gpsimd.collective_compute(
                    "AllGather", mybir.AluOpType.bypass,
                    replica_groups=[list(range(C))],
                    ins=[t2_loc[:]], outs=[t2_gp[:]])
                respace(t2_pad, t2_gp)
                # B init = u2^T (self-loop term for layer 2)
                for j0 in range(0, NJ, 4):
                    pb4 = pst.tile([HID, 4, P], DT.float32, space="PSUM",
                                   tag="pp2")
                    for jj in range(4):
                        nc.tensor.transpose(out=pb4[:, jj, :],
                                            in_=u2[:, j0 + jj, :],
                                            identity=ident_p[:])
                    nc.scalar.activation(
                        out=B[:].rearrange("f (p jj) -> f jj p", jj=NJ)[
                            :, j0:j0 + 4, :],
                        in_=pb4[:], func=mybir.ActivationFunctionType.Copy)
                mark("edge2")
                edge_phase(t2_pad, B[:], idxpool, init_copy=False)
                mark("final")

            # L2 final: y = (B^T @ W2)*dinv + b2 ; log_softmax
            y = con.tile([P, NJ, OUT], DT.float32)
            for j0 in range(0, NJ, 4):
                py4 = pst.tile([P, 4, OUT], DT.float32, space="PSUM", tag="pp")
                for jj in range(4):
                    nc.tensor.matmul(
                        out=py4[:, jj, :],
                        lhsT=B[:].rearrange("f (p jj) -> f p jj",
                                            jj=NJ)[:, :, j0 + jj],
                        rhs=w2_sb[:], start=True, stop=True)
                nc.scalar.activation(out=y[:, j0:j0 + 4, :], in_=py4[:],
                                     func=mybir.ActivationFunctionType.Copy)
            nc.vector.tensor_tensor(
                out=y[:], in0=y[:],
                in1=dinv[:][:, :, None].broadcast_to([P, NJ, OUT]),
                op=mybir.AluOpType.mult)
            nc.vector.tensor_tensor(
                out=y[:], in0=y[:],
                in1=b2b[:][:, None, :].broadcast_to([P, NJ, OUT]),
                op=mybir.AluOpType.add)
            mx = con.tile([P, NJ], DT.float32)
            nc.vector.tensor_reduce(out=mx[:], in_=y[:], op=mybir.AluOpType.max,
                                    axis=mybir.AxisListType.X)
            ys = con.tile([P, NJ, OUT], DT.float32)
            nc.vector.tensor_tensor(
                out=ys[:], in0=y[:],
                in1=mx[:][:, :, None].broadcast_to([P, NJ, OUT]),
                op=mybir.AluOpType.subtract)
            ex = u_st[:, :, :OUT]  # reuse
            nc.scalar.activation(out=ex,
                                 in_=ys[:], func=mybir.ActivationFunctionType.Exp)
            sm = con.tile([P, NJ], DT.float32)
            nc.vector.tensor_reduce(out=sm[:], in_=ex, op=mybir.AluOpType.add,
                                    axis=mybir.AxisListType.X)
            lg = con.tile([P, NJ], DT.float32)
            nc.scalar.activation(out=lg[:], in_=sm[:],
                                 func=mybir.ActivationFunctionType.Ln)
            res = y  # reuse
            nc.vector.tensor_tensor(
                out=res[:], in0=ys[:],
                in1=lg[:][:, :, None].broadcast_to([P, NJ, OUT]),
                op=mybir.AluOpType.subtract)
            nc.sync.dma_start(out=out_t[:], in_=res[:].rearrange("p j o -> p (j o)"))
            mark(None)

    nc.compile()
    return nc


def prepare(x, edge_index, W1, b1v, W2, b2v):
    x = np.asarray(x, np.float32)
    ei = np.asarray(edge_index)
    src = ei[0].astype(np.int64)
    dst = ei[1].astype(np.int64)

    core_data = []
    for c in range(C):
        m = (dst >= c * SL) & (dst < (c + 1) * SL)
        s_c = src[m]
        d_c = dst[m] - c * SL          # local dst id l in [0, 12500)
        res = s_c % NRES               # src%4 (12500%4==0 so local==global)
        win = d_c // W
        pg = win // G
        order = np.lexsort((d_c, win, res, pg))
        core_data.append((s_c[order], d_c[order],
                          res[order], win[order]))

    ntile_rw = np.zeros((NRES, NW), np.int64)
    for c in range(C):
        _, d_c, r_c, w_c = core_data[c]
        for r in range(NRES):
            wcounts = np.bincount(w_c[r_c == r], minlength=NW)
            ntile_rw[r] = np.maximum(ntile_rw[r], (wcounts + P - 1) // P)

    pgs, nt = pg_structure(ntile_rw)
    n_slots = nt * P

    in_maps = []
    for c in range(C):
        s_c, d_c, r_c, w_c = core_data[c]
        idx_flat = np.zeros(n_slots, np.int16)
        dstw = np.full((P, nt), 512.0, np.float32)
        # edges sorted by (pg, r, w, d); groups keyed (pg, r, w)
        pg_c = w_c // G
        grp_id = (pg_c * NRES + r_c) * NW + w_c
        bounds = np.searchsorted(grp_id, np.arange(NPG * NRES * NW + 1))
        t0 = 0
        for pg in range(NPG):
            w0, w1_ = pg * G, min((pg + 1) * G, NW)
            for r in range(NRES):
                for w in range(w0, w1_):
                    g = (pg * NRES + r) * NW + w
                    lo, hi = bounds[g], bounds[g + 1]
                    cnt = hi - lo
                    T = int(ntile_rw[r, w])
                    rows = ((s_c[lo:hi] // SL) * ROWS_L
                            + (s_c[lo:hi] % SL) // NRES)
                    dloc = d_c[lo:hi] - w * W
                    sl = np.zeros(T * P, np.int64)
                    dw = np.full(T * P, 512.0, np.float32)
                    sl[:cnt] = rows
                    dw[:cnt] = dloc
                    idx_flat[t0 * P:(t0 + T) * P] = sl.astype(np.int16)
                    dstw[:, t0:t0 + T] = dw.reshape(T, P).T
                    t0 += T
        assert t0 == nt
        idx_wrapped = np.tile(idx_flat.reshape(n_slots // 16, 16).T, (8, 1)).copy()

        xs = np.zeros((S, F_IN), np.float32)
        xs[:SL] = x[c * SL:(c + 1) * SL]
        # xT columns ordered (j, p) with node l = p*NJ + j
        xT = np.ascontiguousarray(
            xs.reshape(P, NJ, F_IN).transpose(2, 1, 0).reshape(F_IN, S)
        ).astype(np.float16)

        degc = np.bincount(d_c, minlength=S).astype(np.float32) + 1.0
        deg_pj = degc.reshape(P, NJ).copy()            # [p, j] = deg[p*NJ+j]

        in_maps.append({
            "xT": xT,
            "w1": np.asarray(W1, np.float16),
            "b1": np.asarray(b1v, np.float32).reshape(1, HID),
            "w2": np.asarray(W2, np.float32),
            "b2": np.asarray(b2v, np.float32).reshape(1, OUT),
            "deg": deg_pj,
            "idx": idx_wrapped,
            "dstw": dstw.astype(np.float16),
        })
    return ntile_rw, n_slots, in_maps


def kernel(x, edge_index, W1, b1, W2, b2):
    ntile_rw, n_slots, in_maps = prepare(x, edge_index, W1, b1, W2, b2)
    nc = build_program(ntile_rw, n_slots)
    res = run_bass_kernel_spmd(nc, in_maps, core_ids=list(range(C)))
    outs = []
    for c in range(C):
        o = res.results[c]["out"].reshape(P, NJ, OUT)   # [p, j, o], l = p*NJ+j
        o = o.reshape(S, OUT)[:SL]
        outs.append(o)
    return np.concatenate(outs, 0).astype(np.float32)
